# revision 1
# baseline (speedup 1.0000x reference)
"""Distributed Bass kernel for pre-LN multi-head attention on 8 TRN2 NeuronCores.

Problem: x[2, 2048, 1024] -> LayerNorm -> QKV (16 heads x 64) -> softmax(QK^T/8)V
         -> out proj [1024] + bias.

Sharding: core = (batch b, query-block qb) with 4 query blocks of 512 tokens per
batch. Each core receives the FULL batch x[b] (transposed) plus its own query
slice, recomputes LayerNorm + K/V projections for the whole batch locally, and
runs attention for all 16 heads over its 512 queries. No collectives (NEFFs
with collective_compute fail to load on this runtime) and no on-device
transposes (xbar-transpose DMAs serialize against normal DMAs on real HW).

LayerNorm: per-token sums and sums-of-squares are computed on the TensorE with
a ones-vector stationary operand (accumulated over the 8 contraction chunks),
from the transposed activations and a gpsimd-squared copy staged in the
not-yet-written vf/QT tile space — this keeps LN stats off ScalarE, whose real
throughput (~3 cycles/elem, 2.5x the cost model) makes it the kernel-wide
bottleneck via the softmax exp. The per-token affine (rstd, -mean*rstd) is
applied in transposed space via stride-0 broadcast DMAs; ln_scale/ln_bias are
folded into the QKV weights on the host (exact: xn@W = xhat@(diag(s)W) + (b@W),
the latter a constant row added per column). The natural-layout x inputs are
not needed at all, saving 5.2MB of HBM reads per core.

Attention: softmax without max subtraction (scores are O(+-8) for this model's
scale), exp on ScalarE with the 1/sqrt(64) folded into the activation scale,
denominator from a ones-column appended to V (M=65 matmul) riding the attn@V
accumulation. Head-pair row-packing uses both PE row groups for the Dh=64
score matmuls. All matmul operands bf16 (host pre-cast), accumulation f32.

Pipelining: the kernel is a merged projection/attention pipeline. ScalarE's
measured exp throughput (~3 cycles/elem, flat in width and dtype) makes the
softmax stream the per-core floor (~318us), so everything else is arranged to
hide beneath it: score tiles are per-head single-PSUM-bank (exp is flat-rate,
splitting the paired exp is free), which fits projections + scores + attn@V
accumulators in the 8 PSUM banks simultaneously (1 + 3 + 4). The K projection
for head-pair pc+1 and the two V half-projections are emitted just-in-time
inside the attention loop (V-half0 within pair 0, V-half1 within pair 2,
K chunk pc+1 at the midpoint of pair pc), so the TensorE executes them in its
slack while ScalarE streams exps; the 3-deep score pipeline absorbs semaphore
latency between the score fill and its exp. Phase A (LN + Q + first K/V tile)
is the only serial head; dissolving its pool boundary is the documented next
step.
"""

import numpy as np
import ml_dtypes

import concourse.bass as bass
import concourse.mybir as mybir
import concourse.tile as tile
from concourse import bacc
from concourse.bass import ts, ds
from concourse.bass_utils import run_bass_kernel_spmd

B, S, D = 2, 2048, 1024
H, DH = 16, 64
INNER = H * DH  # 1024
N_CORES = 8
QB = 4                 # query blocks per batch
TPC = S // QB          # 512 tokens per core
F32 = mybir.dt.float32
BF16 = mybir.dt.bfloat16
AF = mybir.ActivationFunctionType
OP = mybir.AluOpType

DEBUG = False
PHASES = 4

NT = TPC // 128        # 4 token tiles per query block
NTF = S // 128         # 16 token tiles in the full batch
NDC = D // 128         # 8 contraction chunks over D
NKC = S // 128         # 16 k-position chunks over full sequence
NPAIR = H // 2         # 8 head pairs


def _pe_stats(nc, stps, lnp, xT_sb, ones_t, eps_t, a_dram, c_dram,
              sq_sb, n_groups):
    """Per-token LN stats computed without touching ScalarE's throughput path:
    token sums via ones-vector matmuls on the PE (accumulated over the 8
    contraction chunks), sums of squares the same way from a DVE-squared copy.
    Outputs a=rstd and c=-mean*rstd rows staged to DRAM for the broadcast
    loads in _ln_apply_T."""
    for tg in range(n_groups):
        nc.gpsimd.tensor_tensor(
            sq_sb[:, :, ds(tg * TPC, TPC)], xT_sb[:, :, ds(tg * TPC, TPC)],
            xT_sb[:, :, ds(tg * TPC, TPC)], op=OP.mult)
        ssum = stps.tile([1, TPC], F32, tag="ssum")
        ssq = stps.tile([1, TPC], F32, tag="ssq")
        for dc in range(NDC):
            nc.tensor.matmul(
                ssum[:], ones_t[:], xT_sb[:, dc, ds(tg * TPC, TPC)],
                start=(dc == 0), stop=(dc == NDC - 1))
        for dc in range(NDC):
            nc.tensor.matmul(
                ssq[:], ones_t[:], sq_sb[:, dc, ds(tg * TPC, TPC)],
                start=(dc == 0), stop=(dc == NDC - 1))
        mean = lnp.tile([1, TPC], F32, tag="mean")
        nc.vector.tensor_scalar(mean[:], ssum[:], 1.0 / D, None, op0=OP.mult)
        msq = lnp.tile([1, TPC], F32, tag="msq")
        nc.vector.tensor_tensor(msq[:], mean[:], mean[:], op=OP.mult)
        var = lnp.tile([1, TPC], F32, tag="var")
        nc.vector.scalar_tensor_tensor(
            var[:], ssq[:], 1.0 / D, msq[:], op0=OP.mult, op1=OP.subtract)
        std = lnp.tile([1, TPC], F32, tag="std")
        nc.scalar.activation(std[:], var[:], AF.Sqrt, bias=eps_t[0:1, 0:1])
        rstd = lnp.tile([1, TPC], F32, tag="rstd")
        nc.vector.reciprocal(rstd[:], std[:])
        cb = lnp.tile([1, TPC], F32, tag="cb")
        nc.vector.scalar_tensor_tensor(
            cb[:], mean[:], -1.0, rstd[:], op0=OP.mult, op1=OP.mult)
        nc.sync.dma_start(
            a_dram[ds(tg * TPC, TPC)].rearrange("(o t) -> o t", o=1), rstd[:])
        nc.sync.dma_start(
            c_dram[ds(tg * TPC, TPC)].rearrange("(o t) -> o t", o=1), cb[:])


def _ln_apply_T(nc, tc, lnp, xT_sb, a_dram, c_dram, n_tok):
    """In-place normalize the transposed activations: xT = xT*a + c, with a/c
    broadcast across partitions from DRAM (per 512-token group)."""
    for tg in range(n_tok // TPC):
        a_bc = lnp.tile([128, TPC], F32, tag="a_bc")
        nc.sync.dma_start(
            a_bc[:],
            a_dram[ds(tg * TPC, TPC)].rearrange(
                "(o t) -> o t", o=1)[0:1, :].to_broadcast((128, TPC)))
        c_bc = lnp.tile([128, TPC], F32, tag="c_bc")
        nc.sync.dma_start(
            c_bc[:],
            c_dram[ds(tg * TPC, TPC)].rearrange(
                "(o t) -> o t", o=1)[0:1, :].to_broadcast((128, TPC)))
        a_b = a_bc[:].rearrange("p (o t) -> p o t", o=1).to_broadcast((128, NDC, TPC))
        c_b = c_bc[:].rearrange("p (o t) -> p o t", o=1).to_broadcast((128, NDC, TPC))
        sl = xT_sb[:, :, ds(tg * TPC, TPC)]
        nc.vector.tensor_tensor(sl, sl, a_b, op=OP.mult)
        nc.gpsimd.tensor_tensor(sl, sl, c_b, op=OP.add)


def _attn_chunks(nc, dbg, attp, QT, ktf, vf, avs, pc, kc_lo, kc_hi, scpool):
    h0 = 2 * pc
    for kc in range(kc_lo, kc_hi):
        sc = scpool.tile([128, 2, TPC], F32, tag="sc", name=f"sc{pc}_{kc}")
        for hp in range(2):
            nc.tensor.matmul(
                sc[:, hp, :],
                ktf[ds(hp * 64, 64), pc, ds(kc * 128, 128)],
                QT[ds(hp * 64, 64), pc, :],
                start=True, stop=True)
        ex = attp.tile([128, 2, TPC], BF16, tag="ex")
        nc.scalar.activation(ex[:], sc[:], AF.Exp, scale=0.125)
        if dbg and pc == 0 and kc == 0:
            nc.sync.dma_start(dbg["d_ex0"][:, :, :], ex[:])
        for hp in range(2):
            # V cols + ones col: rows 0:64 = attn out, row 64 = den
            nc.tensor.matmul(
                avs[hp][0:65, :],
                vf[:, kc, ds((h0 + hp) * 65, 65)], ex[:, hp, :],
                start=(kc == 0), stop=(kc == NKC - 1))


def _attn_tail(nc, dbg, rcpp, rdrm, attn_nT, avs, pc):
    # normalize: recip of den row, DMA-broadcast across partitions,
    # folded into the PSUM->SBUF copy
    for hp in range(2):
        rsb = rcpp.tile([128, TPC], F32, tag="rsb")
        nc.vector.reciprocal(rsb[ds(64, 1), :], avs[hp][ds(64, 1), :])
        rdr = rdrm.tile([1, TPC], F32, tag="rdr")
        nc.sync.dma_start(rdr[0:1, :], rsb[ds(64, 1), :])
        rbc = rcpp.tile([64, TPC], F32, tag="rbc")
        nc.sync.dma_start(rbc[:, :], rdr[0:1, :].to_broadcast((64, TPC)))
        if dbg and pc == 0:
            nc.sync.dma_start(dbg["d_rbc0"][ds(hp * 64, 64), :], rbc[:, :])
        if hp == 0:
            nc.vector.scalar_tensor_tensor(
                attn_nT[0:64, pc, :], avs[hp][0:64, :], 1.0, rbc[:],
                op0=OP.mult, op1=OP.mult)
        else:
            tmpn = rcpp.tile([64, TPC], BF16, tag="tmpn")
            nc.vector.scalar_tensor_tensor(
                tmpn[:], avs[hp][0:64, :], 1.0, rbc[:],
                op0=OP.mult, op1=OP.mult)
            nc.sync.dma_start(attn_nT[ds(64, 64), pc, :], tmpn[:])


def _build_iter(nc, tc, ext, it):
    """One full attention forward for this core's shard."""
    (xT_ext, xqT_ext, wqkv_ext, wout_ext,
     qkvb_ext, bout_ext, out_ext) = ext
    dbg = {}
    if DEBUG and it == 0:
        for nm, shp, dt in [("d_xnT", [128, NDC, S], BF16),
                            ("d_QT", [128, NDC, TPC], BF16),
                            ("d_ktf", [128, NDC, S], BF16),
                            ("d_vf", [128, NKC, H * 65], BF16),
                            ("d_attn", [128, NPAIR, TPC], BF16),
                            ("d_ex0", [128, 2, TPC], BF16),
                            ("d_rbc0", [128, TPC], F32)]:
            dbg[nm] = nc.declare_dram_parameter(nm, shp, dt, isOutput=True)

    with tc.tile_pool(name=f"const{it}", bufs=1) as constp, \
         tc.tile_pool(name=f"persist{it}", bufs=1) as pers, \
         tc.tile_pool(name=f"dram{it}", bufs=1, space="DRAM") as dram:

        # ---- constants ----
        eps_t = constp.tile([128, 1], F32)
        nc.vector.memset(eps_t[:], 1e-6)
        bout_bc = constp.tile([128, D], BF16)
        nc.sync.dma_start(
            bout_bc[:],
            bout_ext[:].rearrange("(o d) -> o d", o=1)[0:1, :].to_broadcast((128, D)))
        qkvb_t = constp.tile([128, 24], F32)     # qkv bias row, per-partition form
        nc.sync.dma_start(qkvb_t[:], qkvb_ext[:].rearrange("(c p) -> p c", p=128))
        vb_bc = constp.tile([128, INNER], F32)    # v-bias row broadcast
        nc.sync.dma_start(
            vb_bc[:],
            qkvb_ext[ds(2 * INNER, INNER)].rearrange(
                "(o d) -> o d", o=1)[0:1, :].to_broadcast((128, INNER)))

        # ---- persistent activations ----
        QT = pers.tile([128, NDC, TPC], BF16)         # [qcol-chunk, q] (col=qc*128+p)
        attn_nT = pers.tile([128, NPAIR, TPC], BF16)  # normalized attn out, transposed
        ktf = pers.tile([128, NDC, S], BF16)          # K^T full batch: [col, kpos]
        vf = pers.tile([128, NKC, H * 65], BF16)      # V + ones col: [kpos%128, kc, (h,65)]

        a_dram = dram.tile([S], F32)
        c_dram = dram.tile([S], F32)
        aq_dram = dram.tile([TPC], F32)
        cq_dram = dram.tile([TPC], F32)

        # ====== Phase A: LayerNorm + Q projection + first K/V chunks ======
        # (stps carries the stats rows AND the phase-A projection accumulators;
        # it closes before the merged phase so PSUM frees up for 2+2+4.)
        with tc.tile_pool(name=f"xnt{it}", bufs=1) as xntp, \
             tc.tile_pool(name=f"wq{it}", bufs=1) as wqp:
            xnT = xntp.tile([128, NDC, S], BF16)
            wq = wqp.tile([128, NDC, 3 * INNER], BF16)

            def vproj(kc, nh, pool):
                ps = pool.tile([128, TPC], F32, tag="proj")
                for dc in range(NDC):
                    nc.tensor.matmul(
                        ps[:], xnT[:, dc, ds(kc * 128, 128)],
                        wq[:, dc, ds(2 * INNER + nh * 512, 512)],
                        start=(dc == 0), stop=(dc == NDC - 1))
                nc.vector.scalar_tensor_tensor(
                    vf[:, kc, :].rearrange(
                        "p (h c2) -> p h c2", c2=65)[:, ds(nh * 8, 8), 0:64],
                    ps[:].rearrange("p (h d) -> p h d", d=64), 1.0,
                    vb_bc[:, ds(nh * 512, 512)].rearrange(
                        "p (h d) -> p h d", d=64),
                    op0=OP.mult, op1=OP.add)

            def kproj(qc, pool):
                for tg in range(QB):
                    ps = pool.tile([128, TPC], F32, tag="proj")
                    for dc in range(NDC):
                        nc.tensor.matmul(
                            ps[:], wq[:, dc, ds(INNER + qc * 128, 128)],
                            xnT[:, dc, ds(tg * TPC, TPC)],
                            start=(dc == 0), stop=(dc == NDC - 1))
                    nc.vector.tensor_scalar(
                        ktf[:, qc, ds(tg * TPC, TPC)], ps[:], 1.0,
                        qkvb_t[:, 8 + qc:9 + qc], op0=OP.mult, op1=OP.add)

            with tc.tile_pool(name=f"xqt{it}", bufs=1) as xqtp, \
                 tc.tile_pool(name=f"ln{it}", bufs=2) as lnp, \
                 tc.tile_pool(name=f"stps{it}", bufs=2, space="PSUM") as stps:
                xqT = xqtp.tile([128, NDC, TPC], BF16)
                ones_t = constp.tile([128, 1], BF16)
                nc.vector.memset(ones_t[:], 1.0)
                nc.sync.dma_start(
                    xqT[:], xqT_ext[:, :].rearrange("(c p) t -> p c t", p=128))
                for tg in range(QB):
                    nc.sync.dma_start(
                        xnT[:, :, ds(tg * TPC, TPC)],
                        xT_ext[:, ds(tg * TPC, TPC)].rearrange(
                            "(c p) t -> p c t", p=128))
                nc.sync.dma_start(
                    wq[:], wqkv_ext[:, :].rearrange("(c p) n -> p c n", p=128))
                # QT / vf free space double as scratch for the squared copies
                _pe_stats(nc, stps, lnp, xqT, ones_t, eps_t, aq_dram, cq_dram,
                          QT[:], 1)
                _ln_apply_T(nc, tc, lnp, xqT, aq_dram, cq_dram, TPC)
                sqf = vf[:].rearrange("p a c -> p (a c)")[:, 0:NDC * S] \
                    .rearrange("p (g t) -> p g t", t=S)
                _pe_stats(nc, stps, lnp, xnT, ones_t, eps_t, a_dram, c_dram,
                          sqf, QB)
                _ln_apply_T(nc, tc, lnp, xnT, a_dram, c_dram, S)

                # Q^T for own queries (phase-A psum)
                for qc in range(NDC):
                    ps = stps.tile([128, TPC], F32, tag="proj")
                    for dc in range(NDC):
                        nc.tensor.matmul(
                            ps[:], wq[:, dc, ts(qc, 128)], xqT[:, dc, :],
                            start=(dc == 0), stop=(dc == NDC - 1))
                    nc.vector.tensor_scalar(
                        QT[:, qc, :], ps[:], 1.0, qkvb_t[:, qc:qc + 1],
                        op0=OP.mult, op1=OP.add)
                # ones columns for the in-matmul softmax denominator
                for kc in range(NKC):
                    nc.vector.memset(
                        vf[:, kc, :].rearrange(
                            "p (h c2) -> p h c2", c2=65)[:, :, 64:65],
                        1.0)
                # first K chunk + first V tile so attention can start
                kproj(0, stps)
                vproj(0, 0, stps)

            # ====== Merged phase: attention streams on ACT while the PE
            # finishes the remaining K/V projections in its slack ======
            with tc.tile_pool(name=f"att{it}", bufs=3) as attp, \
                 tc.tile_pool(name=f"rcp{it}", bufs=2) as rcpp, \
                 tc.tile_pool(name=f"wo{it}", bufs=1) as wop, \
                 tc.tile_pool(name=f"oac{it}", bufs=1) as oacp, \
                 tc.tile_pool(name=f"rdrm{it}", bufs=3, space="DRAM") as rdrm, \
                 tc.tile_pool(name=f"scps{it}", bufs=3, space="PSUM") as scps, \
                 tc.tile_pool(name=f"avps{it}", bufs=4, space="PSUM") as avps, \
                 tc.tile_pool(name=f"qkvps{it}", bufs=1, space="PSUM") as qkvps:
                wo = wop.tile([128, NDC, D], BF16)
                nc.sync.dma_start(
                    wo[:], wout_ext[:, :].rearrange("(c p) n -> p c n", p=128))
                oacc = oacp.tile([128, NT, D], BF16)
                for pc in range(NPAIR):
                    h0 = 2 * pc
                    avs = [avps.tile([128, TPC], F32, tag="av",
                                     name=f"av{pc}_{hp}") for hp in range(2)]
                    for kc in range(NKC):
                        # JIT projections woven into the ACT-bound stream
                        if pc == 0 and kc >= 1:
                            vproj(kc, 0, qkvps)
                        if pc == 2:
                            vproj(kc, 1, qkvps)
                        if kc == 8 and pc < NPAIR - 1:
                            kproj(pc + 1, qkvps)
                        for hp in range(2):
                            sc = scps.tile([128, TPC], F32, tag="sc",
                                           name=f"sc{pc}_{kc}_{hp}")
                            nc.tensor.matmul(
                                sc[:],
                                ktf[ds(hp * 64, 64), pc, ds(kc * 128, 128)],
                                QT[ds(hp * 64, 64), pc, :],
                                start=True, stop=True)
                            ex = attp.tile([128, TPC], BF16, tag="ex")
                            nc.scalar.activation(ex[:], sc[:], AF.Exp,
                                                 scale=0.125)
                            nc.tensor.matmul(
                                avs[hp][0:65, :],
                                vf[:, kc, ds((h0 + hp) * 65, 65)], ex[:],
                                start=(kc == 0), stop=(kc == NKC - 1))
                    _attn_tail(nc, dbg, rcpp, rdrm, attn_nT, avs, pc)
                    if pc in (3, 4):
                        for tt in range((pc - 3) * 2, (pc - 2) * 2):
                            for nh in range(2):
                                po = avps.tile([128, TPC], F32, tag="av",
                                               name=f"poA{tt}_{nh}")
                                for pj in range(4):
                                    nc.tensor.matmul(
                                        po[:], attn_nT[:, pj, ts(tt, 128)],
                                        wo[:, pj, ds(nh * 512, 512)],
                                        start=(pj == 0), stop=(pj == 3))
                                nc.vector.scalar_tensor_tensor(
                                    oacc[:, tt, ds(nh * 512, 512)], po[:], 1.0,
                                    bout_bc[:, ds(nh * 512, 512)],
                                    op0=OP.mult, op1=OP.add)

                # ====== out projection (pairs 4-7 + merge) ======
                with tc.tile_pool(name=f"out{it}", bufs=1) as outp:
                    for tt in range(NT):
                        o_nat = outp.tile([128, D], F32, tag="o_nat")
                        for nh in range(2):
                            po = avps.tile([128, TPC], F32, tag="av",
                                           name=f"poB{tt}_{nh}")
                            for pj in range(4, NPAIR):
                                nc.tensor.matmul(
                                    po[:], attn_nT[:, pj, ts(tt, 128)],
                                    wo[:, pj, ds(nh * 512, 512)],
                                    start=(pj == 4), stop=(pj == NPAIR - 1))
                            nc.vector.scalar_tensor_tensor(
                                o_nat[:, ds(nh * 512, 512)], po[:], 1.0,
                                oacc[:, tt, ds(nh * 512, 512)],
                                op0=OP.mult, op1=OP.add)
                        nc.sync.dma_start(out_ext[ts(tt, 128), :], o_nat[:])

        if dbg:
            nc.sync.dma_start(dbg["d_QT"][:, :, :], QT[:])
            nc.sync.dma_start(dbg["d_ktf"][:, :, :], ktf[:])
            nc.sync.dma_start(dbg["d_vf"][:, :, :], vf[:])
            nc.sync.dma_start(dbg["d_attn"][:, :, :], attn_nT[:])


def build_bass(n_iters=1):
    nc = bacc.Bacc(None, num_devices=N_CORES)
    xT_ext = nc.declare_dram_parameter("xT", [D, S], BF16, isOutput=False)
    xqT_ext = nc.declare_dram_parameter("xqT", [D, TPC], BF16, isOutput=False)
    wqkv_ext = nc.declare_dram_parameter("w_qkv", [D, 3 * INNER], BF16, isOutput=False)
    wout_ext = nc.declare_dram_parameter("w_out", [INNER, D], BF16, isOutput=False)
    qkvb_ext = nc.declare_dram_parameter("qkv_bias", [3 * INNER], F32, isOutput=False)
    bout_ext = nc.declare_dram_parameter("b_out", [D], BF16, isOutput=False)
    out_ext = nc.declare_dram_parameter("out", [TPC, D], F32, isOutput=True)
    ext = (xT_ext, xqT_ext, wqkv_ext, wout_ext,
           qkvb_ext, bout_ext, out_ext)
    with tile.TileContext(nc) as tc:
        for it in range(n_iters):
            _build_iter(nc, tc, ext, it)
    nc.finalize()
    return nc


def make_in_maps(x, ln_scale, ln_bias, w_qkv, w_out, b_out):
    bf = ml_dtypes.bfloat16
    lns = np.asarray(ln_scale, np.float32)
    lnb = np.asarray(ln_bias, np.float32)
    # fold ln scale into the qkv weights; ln bias becomes a constant qkv row
    wq_s = (np.asarray(w_qkv, np.float32) * lns[:, None]).astype(bf)
    qkvb = (lnb @ np.asarray(w_qkv, np.float32)).astype(np.float32)
    wo = np.ascontiguousarray(w_out).astype(bf)
    bo = np.ascontiguousarray(b_out).astype(bf)
    xbf = [np.ascontiguousarray(x[b]).astype(bf) for b in range(B)]
    xTbf = [np.ascontiguousarray(xbf[b].T) for b in range(B)]
    in_maps = []
    for core in range(N_CORES):
        b, qb = core // QB, core % QB
        in_maps.append({
            "xT": xTbf[b],
            "xqT": np.ascontiguousarray(xTbf[b][:, qb * TPC:(qb + 1) * TPC]),
            "w_qkv": wq_s, "w_out": wo, "qkv_bias": qkvb, "b_out": bo,
        })
    return in_maps


_CACHED_NC = None


def kernel(x, ln_scale, ln_bias, w_qkv, w_out, b_out):
    global _CACHED_NC
    if _CACHED_NC is None:
        _CACHED_NC = build_bass(n_iters=1)
    in_maps = make_in_maps(x, ln_scale, ln_bias, w_qkv, w_out, b_out)
    res = run_bass_kernel_spmd(_CACHED_NC, in_maps, list(range(N_CORES)))
    out = np.empty((B, S, D), np.float32)
    for core in range(N_CORES):
        b, qb = core // QB, core % QB
        out[b, qb * TPC:(qb + 1) * TPC, :] = res.results[core]["out"]
    return out

